# revision 32
# baseline (speedup 1.0000x reference)
"""Causal dot-product attention (s=2048, b=4, h=16, d=128) on 8 TRN2 NeuronCores.

Sharding: batch*heads (64 pairs) split across 8 cores -> 8 (b,h) pairs per core.
Core c handles b = c // 2, heads h in [(c%2)*8, (c%2)*8 + 8).

Per-core kernel (Bass/Tile), per head:
  S^T[sk, sq] = K^T_j(stationary) . Q^T(moving)   (fp16 in, fp32 PSUM out)
  E = exp(S^T * 1/sqrt(d)), split across TWO engines per group ("ride-along"):
      ScalarE ACTIVATE exps slot 0 of each 2-j-tile group; for every
      non-diagonal group of i5>=1 the DVE exps slot 1 concurrently with a
      Schraudolph int16 approximation E ~= bitcast_fp16(int16(s*A + B))
      (rms rel err 1.8%; those tiles carry small softmax weight -> ~0.4%
      output error).  ScalarE was the critical engine at (N+352)/1.2ns per
      ACTIVATE; this moves ~30% of its columns to the underused DVE.
      Hard-won structural rules (each violation cost 10-90us):
       - the trick writes its OWN tile (etr): two engines concurrently
         writing one SBUF tile slows the whole chip ~20%
       - tricks are emitted right behind their QK matmuls so they never
         queue behind ACT-dependent work in the in-order DVE FIFO
       - acts run at ONE group of pipeline delay (s_ps recycling stays
         act-paced through the 2-buffer PSUM pool), PV matmuls at THREE
         groups, so PE reaches each group's PV long after its exps landed
       - the epilogue is split into four ~0.5us DVE pieces drained one per
         group; a monolithic 1.7us epilogue lump in the DVE FIFO stalls
         the tricks (and thus QK) at every i5 seam
  causal: skip sk>sq blocks/columns; triangular mask of the diagonal
          128-wide subtile into a separate em tile via gpsimd.affine_select
          (GPSIMD is otherwise idle; keeps the masks out of the DVE FIFO)
  ctx[sq, 0:128] + rowsum[sq] (col 128) = sum_j E_j^T(stationary) . [V_j | 1]
  out = ctx * (1/rowsum)     (DVE per-ctx-bank batched reciprocal +
                              per-partition scalar multiply)

Host-side layout prep: Q and K are transposed to [head, d, s], concatenated,
and cast to fp16 (fp32 matmuls cost two PE passes; fp16 also enables fast
weight load).  V is cast to fp16 with the softmax-denominator ones-column
baked in.  One DMA each per head -> single-semaphore waits (walrus rejects
instructions carrying >1 sync wait; Bacc legalizes the rest via event sems).
"""

import sys

if "/opt/trn_rl_repo" not in sys.path:
    sys.path.insert(0, "/opt/trn_rl_repo")

import numpy as np

import concourse.bacc as bacc
import concourse.bass as bass
import concourse.mybir as mybir
import concourse.tile as tile
from concourse.bass_utils import run_bass_kernel_spmd

S, B, H, D = 2048, 4, 16, 128
N_CORES = 8
HPC = (B * H) // N_CORES  # heads per core = 8
SCALE = 1.0 / float(np.sqrt(128.0))

SQ_BLK = 512  # sq block width per j-tile matmul
N_I = S // SQ_BLK  # 4 sq blocks per head
N_SK = S // 128  # 16 sk tiles per head
VW = 129  # V tile width incl. ones column

# Schraudolph fp16 exp: exp(SCALE*s) ~= bitcast_fp16(int16(s*EXP_A + EXP_B));
# EXP_B zeroes the mean relative error over f~U[0,1] (rms 1.77%).
EXP_A = float(SCALE * np.log2(np.e) * 1024.0)
EXP_B = 15300.62


# in-slot first live trick column of slot 1, per i5: i5=1 seams have the
# least PE cover, so its DVE share is smallest (act 768 / trick 256); deeper
# i5 runs use act 640 / trick 384
TRICK_C0 = {1: 256, 2: 128, 3: 128}


def tricked(i5, p):
    # slot-1 of every non-diagonal group of i5>=1 gets the DVE exp while
    # ScalarE exps slot 0 of the same group in parallel ("ride-along"): the
    # s_ps release stays ACT-paced (the trick is shorter and starts earlier),
    # so the QK pipeline never blocks on the DVE.  i5=0 is excluded: its
    # early rows have few keys, so approximating them costs real accuracy.
    # also the {t3,t0} diagonal group's slot 1 (t0): its big ACTIVATE
    # paced the i5-transition chain through the 2-buffer score pool
    return (p < 2 * i5 or p == 2 * i5) and i5 >= 1


def build_nc():
    nc = bacc.Bacc()
    qk = nc.dram_tensor("qk", [HPC, D, 2 * S], mybir.dt.float16, kind="ExternalInput")
    v = nc.dram_tensor("v", [HPC, N_SK, 128, VW], mybir.dt.float16, kind="ExternalInput")
    out = nc.dram_tensor("out", [S, HPC * D], mybir.dt.float32, kind="ExternalOutput")

    with tile.TileContext(nc) as tc:
        with (
            tc.tile_pool(name="const", bufs=1) as constp,
            tc.tile_pool(name="qkp", bufs=2) as qkp,
            tc.tile_pool(name="vp", bufs=3) as vpool,
            tc.tile_pool(name="e", bufs=8) as ep,
            tc.tile_pool(name="stage", bufs=3) as stagep,
            tc.tile_pool(name="rec", bufs=8) as recp,
            tc.tile_pool(name="etr", bufs=8) as etrp,
            tc.tile_pool(name="em", bufs=8) as emp,
            tc.tile_pool(name="ps_s", bufs=2, space="PSUM") as ps_s,
            tc.tile_pool(name="ps_c", bufs=2, space="PSUM") as ps_c,
        ):
            # tiny dummy exp: triggers the one-time ~2.7us ACT table load
            # during the DMA prologue instead of before the first real exp
            warm = constp.tile([1, 8], mybir.dt.float32, name="warm")
            nc.vector.memset(warm[:], 0.0)
            nc.scalar.activation(
                warm[:],
                warm[:],
                mybir.ActivationFunctionType.Exp,
                scale=SCALE,
            )

            started_heads = set()
            started_i5 = set()
            vdummy_done = set()
            pending_epi = []
            ctx_holder = {}
            staged_holder = {}
            qk_holder = {}
            v_holder = {}

            def start_head(hh):
                qk_sb = qkp.tile([128, 2 * S], mybir.dt.float16, tag="qk", name="qk_sb")
                qk_holder[hh] = qk_sb
                v_sb = vpool.tile([128, N_SK * VW], mybir.dt.float16, tag="v", name="v_sb")
                v_holder[hh] = v_sb
                qk4 = qk_sb.rearrange("p (b c) -> p b c", c=SQ_BLK)
                qk4s = qk[hh, :, :].rearrange("p (b c) -> p b c", c=SQ_BLK)
                v3 = v_sb.rearrange("p (j e) -> p j e", e=VW)
                v3s = v[hh, :, :, :].rearrange("j p e -> p j e")
                if hh == 0:
                    # head 0 has no prefetch window: stream qk in compute
                    # order, one block ahead of the matching v tiles (the exp
                    # chain only waits on qk; PE's v wait is absorbed late)
                    nc.sync.dma_start(out=qk4[:, 0 :: N_I, :], in_=qk4s[:, 0 :: N_I, :])
                    for b in range(1, N_I):
                        nc.sync.dma_start(
                            out=qk4[:, b :: N_I, :], in_=qk4s[:, b :: N_I, :]
                        )
                        nc.sync.dma_start(
                            out=v3[:, 4 * (b - 1) : 4 * b, :],
                            in_=v3s[:, 4 * (b - 1) : 4 * b, :],
                        )
                    nc.sync.dma_start(
                        out=v3[:, 4 * (N_I - 1) :, :], in_=v3s[:, 4 * (N_I - 1) :, :]
                    )
                else:
                    # later heads are fully prefetched during the previous head
                    nc.sync.dma_start(out=qk_sb[:], in_=qk[hh, :, :])
                    nc.sync.dma_start(out=v3, in_=v3s)
                staged_holder[hh] = stagep.tile(
                    [128, N_SK * D], mybir.dt.float32, tag="o", name="staged"
                )

            def start_i5(hh, i5):
                ctx_ab = [
                    ps_c.tile(
                        [128, 2 * VW], mybir.dt.float32, tag=f"ctx{t}", name=f"ctx{t}"
                    )
                    for t in range(2)
                ]
                ctx_holder[(hh, i5)] = (
                    ctx_ab,
                    [
                        ctx_ab[tt // 2][:, (tt % 2) * VW : (tt % 2 + 1) * VW]
                        for tt in range(4)
                    ],
                )

            def group_js(i5, g):
                # full groups pair adjacent j; the four diagonal j-tiles are
                # cross-paired (highest-c0 tile in quarter 0, lowest in
                # quarter 1) so the exp range [c0_q0 : 1024] skips almost all
                # causally-dead columns
                if g < 2 * i5:
                    return (2 * g, 2 * g + 1)
                if g == 2 * i5:
                    return (4 * i5 + 3, 4 * i5)
                return (4 * i5 + 2, 4 * i5 + 1)

            def emit_qk(hh, i5, p):
                if hh not in started_heads:
                    start_head(hh)
                    started_heads.add(hh)
                if hh + 1 < HPC and hh + 1 not in started_heads:
                    # issue the next head's DMAs a full head ahead
                    start_head(hh + 1)
                    started_heads.add(hh + 1)
                if (hh, i5) not in started_i5:
                    start_i5(hh, i5)
                    started_i5.add((hh, i5))
                qk_sb = qk_holder[hh]
                s_ps = ps_s.tile(
                    [128, 2 * SQ_BLK], mybir.dt.float32, tag="s", name="s_ps"
                )
                for q, j in enumerate(group_js(i5, p)):
                    t = j - 4 * i5
                    c0 = 128 * t if t > 0 else 0
                    nc.tensor.matmul(
                        s_ps[:, q * SQ_BLK + c0 : (q + 1) * SQ_BLK],
                        qk_sb[:, S + j * 128 : S + (j + 1) * 128],
                        qk_sb[:, i5 * SQ_BLK + c0 : (i5 + 1) * SQ_BLK],
                        start=True,
                        stop=True,
                    )
                e_sb = ep.tile(
                    [128, 2 * SQ_BLK], mybir.dt.float16, tag="e", name="e_sb"
                )
                etr = None
                if tricked(i5, p):
                    # Schraudolph exp of slot 1 on DVE, emitted right behind
                    # its QK matmuls: e = bitcast_fp16(int16(s*A + B)).
                    # Its OWN tile: concurrent writes by two engines into one
                    # SBUF tile (ACT does slot 0 at the same time) serialize
                    # chip-wide.
                    etr = etrp.tile(
                        [128, SQ_BLK], mybir.dt.float16, tag="etr", name="etr"
                    )
                    c0s1 = TRICK_C0[i5]
                    nc.vector.tensor_scalar(
                        etr[:, c0s1:SQ_BLK].bitcast(mybir.dt.int16),
                        s_ps[:, SQ_BLK + c0s1 : 2 * SQ_BLK],
                        EXP_A,
                        EXP_B,
                        mybir.AluOpType.mult,
                        mybir.AluOpType.add,
                    )
                return s_ps, (e_sb, etr)

            def emit_act(hh, i5, p, s_ps, ebufs):
                # the ScalarE exp (and the diagonal-mask selects that feed
                # off it) run at ONE group of pipeline delay, keeping s_ps
                # recycling act-paced exactly like the baseline
                e_sb, etr = ebufs
                js = group_js(i5, p)
                t_q0 = js[0] - 4 * i5
                lo = 128 * t_q0 if t_q0 > 0 else 0  # first valid col of group
                # split point 640/384: ScalarE had idle slack while the DVE
                # (trick + epilogue piece) slightly exceeded the per-group PE
                # budget, accumulating slips that stalled QK via s_ps reuse
                hi = SQ_BLK + TRICK_C0[i5] if tricked(i5, p) else 2 * SQ_BLK
                nc.scalar.activation(
                    e_sb[:, lo:hi],
                    s_ps[:, lo:hi],
                    mybir.ActivationFunctionType.Exp,
                    scale=SCALE,
                )
                ems = {}
                for q, j in enumerate(js):
                    t = j - 4 * i5
                    if t >= 0:
                        c0 = 128 * t
                        off = q * SQ_BLK
                        # masked diagonal subtile goes to its OWN tile (a
                        # second engine mutating e_sb while PE reads it
                        # serializes chip-wide), on the otherwise-idle GPSIMD
                        # engine: keep where col >= row
                        em = emp.tile(
                            [128, 128], mybir.dt.float16, tag="em", name="em"
                        )
                        nc.gpsimd.affine_select(
                            out=em[:],
                            in_=e_sb[:, off + c0 : off + c0 + 128],
                            compare_op=mybir.AluOpType.is_ge,
                            fill=0.0,
                            base=0,
                            pattern=[[1, 128]],
                            channel_multiplier=-1,
                        )
                        ems[j] = em
                return ems

            def emit_pv(hh, i5, p, ebufs, ems):
                # PV runs at TWO groups of pipeline delay so the DVE
                # Schraudolph exps (and the masks) are long done when PE
                # program order reaches the matmuls that read them
                e_sb, etr = ebufs
                v_sb = v_holder[hh]
                ctx_t = ctx_holder[(hh, i5)][1]
                js = group_js(i5, p)
                if hh not in vdummy_done:
                    # absorb the v-DMA wait on PE right before the head's
                    # first PV matmul (scribbles on ctx, which the j=0
                    # start=True matmul then resets)
                    vdummy_done.add(hh)
                    nc.tensor.matmul(
                        ctx_t[0][0:1, 0:8],
                        v_sb[:, 0:1],
                        v_sb[:, 0:8],
                        start=True,
                        stop=True,
                        skip_group_check=True,
                    )
                # ascending-j emission keeps j=0's bank-clearing start=True
                # matmuls ahead of every other writer of the same psum bank
                for q, j in sorted(enumerate(js), key=lambda qj: qj[1]):
                    t = j - 4 * i5
                    c0 = 128 * t if t > 0 else 0
                    off = q * SQ_BLK
                    # start=True clears the WHOLE psum bank, so only the
                    # bank-first accumulator (tt 0 / 2) may carry it; its
                    # bank-mate's first matmul relies on has_written=0 ->
                    # plain write semantics.
                    # per-tt last-emitted contributor in ascending-j,
                    # cross-paired order: tt0 -> j=4*i5, tt1 -> +1, tt2/tt3 -> +2
                    stop_j = (4 * i5, 4 * i5 + 1, 4 * i5 + 2, 4 * i5 + 2)
                    for tt in range(max(t, 0), 4):
                        if t >= 0 and tt == t:
                            lhs = ems[j][:]
                        elif (
                            etr is not None
                            and q == 1
                            and tt * 128 >= TRICK_C0[i5]
                        ):
                            lhs = etr[:, tt * 128 : (tt + 1) * 128]
                        else:
                            lhs = e_sb[:, off + tt * 128 : off + (tt + 1) * 128]
                        nc.tensor.matmul(
                            ctx_t[tt][:],
                            lhs,
                            v_sb[:, j * VW : (j + 1) * VW],
                            start=(j == 0 and tt % 2 == 0),
                            stop=(j == stop_j[tt]),
                            skip_group_check=True,
                        )
                # the i5's epilogue is split into four ~0.5us DVE pieces and
                # drained ONE PIECE PER GROUP: a monolithic ~1.7us epilogue
                # in the DVE FIFO delays the next groups' Schraudolph exps
                # (which gate s_ps reuse) by more than the PE slack per group
                if pending_epi:
                    pending_epi.pop(0)()
                if p == 2 * (i5 + 1) - 1:
                    shared = {}

                    def rec_ts(hh, i5, tt, shared):
                        ctx_ab, ctx_t = ctx_holder[(hh, i5)]
                        staged = staged_holder[hh]
                        ab = tt // 2
                        if tt % 2 == 0:
                            rec = recp.tile(
                                [128, 2], mybir.dt.float32, tag="rec", name="rec"
                            )
                            nc.vector.reciprocal(
                                rec[:],
                                ctx_ab[ab].rearrange(
                                    "p (two c) -> p two c", c=VW
                                )[:, :, 128],
                            )
                            shared[ab] = rec
                        nc.vector.tensor_scalar_mul(
                            staged[:, (i5 * 4 + tt) * D : (i5 * 4 + tt + 1) * D],
                            ctx_t[tt][:, 0:128],
                            shared[ab][:, tt % 2 : tt % 2 + 1],
                        )
                        if tt == 3:
                            nc.sync.dma_start(
                                out=out[
                                    i5 * SQ_BLK : (i5 + 1) * SQ_BLK,
                                    hh * D : (hh + 1) * D,
                                ].rearrange("(i p) d -> p i d", p=128),
                                in_=staged.rearrange("p (i d) -> p i d", d=D)[
                                    :, i5 * 4 : (i5 + 1) * 4, :
                                ],
                            )

                    for tt in range(4):
                        pending_epi.append(
                            lambda hh=hh, i5=i5, tt=tt, shared=shared: rec_ts(
                                hh, i5, tt, shared
                            )
                        )

            groups = [
                (hh, i5, p)
                for hh in range(HPC)
                for i5 in range(N_I)
                for p in range(2 * (i5 + 1))
            ]
            # split software pipeline: QK(g) ... act(g-1) ... PV(g-2).
            # acts at one group of delay keep s_ps recycling act-paced (a
            # delayed act would serialize QK through the 2-buffer pool); PV
            # at two groups gives the DVE tricks slack.  Uniform PV delay
            # keeps the start=True/stop ordering intact.
            actq = []
            pvq = []
            for g in groups:
                s_ps, ebufs = emit_qk(*g)
                actq.append((g, s_ps, ebufs))
                if len(actq) > 1:
                    g0, s0, b0 = actq.pop(0)
                    ems = emit_act(*g0, s0, b0)
                    pvq.append((g0, b0, ems))
                if len(pvq) > 2:
                    g0, b0, ems = pvq.pop(0)
                    emit_pv(g0[0], g0[1], g0[2], b0, ems)
            for g0, s0, b0 in actq:
                ems = emit_act(*g0, s0, b0)
                pvq.append((g0, b0, ems))
            for g0, b0, ems in pvq:
                emit_pv(g0[0], g0[1], g0[2], b0, ems)
            while pending_epi:
                pending_epi.pop(0)()
    nc.compile()
    return nc


_NC_CACHE = None


def _get_nc():
    global _NC_CACHE
    if _NC_CACHE is None:
        _NC_CACHE = build_nc()
    return _NC_CACHE


def _make_in_maps(query_layer, key_layer, value_layer):
    q = np.asarray(query_layer)
    k = np.asarray(key_layer)
    v = np.asarray(value_layer)
    in_maps = []
    for c in range(N_CORES):
        b = c // 2
        h0 = (c % 2) * HPC
        qkc = np.empty((HPC, D, 2 * S), dtype=np.float16)
        # [s, h, d] -> [h, d, s]
        qkc[:, :, :S] = q[:, b, h0 : h0 + HPC, :].transpose(1, 2, 0)
        qkc[:, :, S:] = k[:, b, h0 : h0 + HPC, :].transpose(1, 2, 0)
        # [s, h, d] -> [h, j, p, d] + ones column -> fp16
        vc = np.ones((HPC, N_SK, 128, VW), dtype=np.float16)
        vc[:, :, :, :D] = (
            v[:, b, h0 : h0 + HPC, :]
            .transpose(1, 0, 2)
            .reshape(HPC, N_SK, 128, D)
            .astype(np.float16)
        )
        in_maps.append({"qk": qkc, "v": vc})
    return in_maps


def run_spmd(in_maps, **kwargs):
    nc = _get_nc()
    return run_bass_kernel_spmd(nc, in_maps, core_ids=list(range(N_CORES)), **kwargs)


def kernel(query_layer, key_layer, value_layer):
    in_maps = _make_in_maps(query_layer, key_layer, value_layer)
    res = run_spmd(in_maps)
    full = np.empty((S, B, H * D), dtype=np.float32)
    for c in range(N_CORES):
        b = c // 2
        h0 = (c % 2) * HPC
        full[:, b, h0 * D : (h0 + HPC) * D] = res.results[c]["out"]
    return full

